# revision 1
# baseline (speedup 1.0000x reference)
"""Trainium2 Bass kernel: Sudoku information gain H(before) - H(after).

Self-contained: builds one SPMD Bass/Tile program, shards the batch
across 8 NeuronCores (pure data parallel), runs via
run_bass_kernel_spmd, and reassembles the full [B] output.

Algorithm per 9x9 grid (values 0..9, 0 = empty):
  encode each cell x as e = 1024 >> x on ScalarE (Exp activation):
    bit 10 <=> empty, bit (10-v) <=> value v.
  Bitwise-OR reductions of e give row/col/box presence masks (VectorE,
  uint16, mostly 2x-mode strided ops).  Per cell m = row|col|box|e_own.
  A SWAR popcount (pairs -> base-16 fields -> mod 15 digit sum) plus
  the own-empty flag folds into u with Ln(u+1) = ln(max(9-forbidden,1))
  for empty cells and 0 for filled ones (ScalarE Ln).  A per-grid
  add-reduce and 1/ln2 scale produce H; before/after are subtracted
  on-chip.
"""

import math
from contextlib import ExitStack

import numpy as np

import concourse.bass as bass
import concourse.bacc as bacc
import concourse.tile as tile
from concourse import mybir
from concourse.alu_op_type import AluOpType
from concourse.bass_utils import run_bass_kernel_spmd

F32 = mybir.dt.float32
U16 = mybir.dt.uint16
I16 = mybir.dt.int16

LN2 = math.log(2.0)
LOG1024 = math.log(1024.0)
EPS = 1e-5

OR = AluOpType.bitwise_or
AND = AluOpType.bitwise_and
ADD = AluOpType.add
SUB = AluOpType.subtract
MULT = AluOpType.mult
MAX = AluOpType.max
SHR = AluOpType.logical_shift_right
MOD = AluOpType.mod

N_CORES = 8
BATCH = 262144
PER_CORE = BATCH // N_CORES  # 32768
F = 16  # grids per partition per tile
USE_MOD = False  # AluOpType.mod is not a valid DVE ISA op
USE_GPSIMD = False  # Pool engine has no bitwise ops
GPSIMD_TREE = False
GPSIMD_ADDS = False  # run arithmetic adds on the Pool engine
MOD_BIAS = -0.4999  # subtracted before the int16 convert in floor(c/15)


def _masks_and_m(nc, wp, e, F):
    ve = e[:]
    e4 = ve.rearrange("p (f r c) -> p f r c", f=F, r=9, c=9)
    e5 = ve.rearrange("p (f b i c) -> p f b i c", f=F, b=3, i=3, c=9)
    gv = nc.gpsimd if USE_GPSIMD else nc.vector

    t3 = wp.tile([128, F * 27], U16, tag="t3")
    t3v = t3[:].rearrange("p (f r b) -> p f r b", f=F, r=9, b=3)
    nc.vector.tensor_tensor(t3v, e4[:, :, :, 0:3], e4[:, :, :, 3:6], op=OR)
    nc.vector.tensor_tensor(t3v, t3v, e4[:, :, :, 6:9], op=OR)

    row = wp.tile([128, F * 9], U16, tag="row")
    rv = row[:].rearrange("p (f r) -> p f r", f=F, r=9)
    t3b = t3[:].rearrange("p (f r b) -> p f r b", f=F, r=9, b=3)
    gv.tensor_tensor(rv, t3b[:, :, :, 0], t3b[:, :, :, 1], op=OR)
    gv.tensor_tensor(rv, rv, t3b[:, :, :, 2], op=OR)

    bcol = wp.tile([128, F * 27], U16, tag="bcol")
    bv = bcol[:].rearrange("p (f b c) -> p f b c", f=F, b=3, c=9)
    nc.vector.tensor_tensor(bv, e5[:, :, :, 0, :], e5[:, :, :, 1, :], op=OR)
    nc.vector.tensor_tensor(bv, bv, e5[:, :, :, 2, :], op=OR)

    col = wp.tile([128, F * 9], U16, tag="col")
    cv = col[:].rearrange("p (f c) -> p f c", f=F, c=9)
    bc3 = bcol[:].rearrange("p (f b c) -> p f b c", f=F, b=3, c=9)
    nc.vector.tensor_tensor(cv, bc3[:, :, 0, :], bc3[:, :, 1, :], op=OR)
    nc.vector.tensor_tensor(cv, cv, bc3[:, :, 2, :], op=OR)

    box = wp.tile([128, F * 9], U16, tag="box")
    xv = box[:].rearrange("p (f b k) -> p f b k", f=F, b=3, k=3)
    bc4 = bcol[:].rearrange("p (f b k i) -> p f b k i", f=F, b=3, k=3, i=3)
    gv.tensor_tensor(xv, bc4[:, :, :, :, 0], bc4[:, :, :, :, 1], op=OR)
    gv.tensor_tensor(xv, xv, bc4[:, :, :, :, 2], op=OR)

    # NOTE: no bit-10 cleanup needed on the unit masks — the SWAR masks
    # (0x155 on m>>1, 0x55 on m>>2) never look at bit 10, and the
    # own-cell empty flag comes straight from e.
    # ISA limit: tensor ops take at most 3 free dims per AP, so the
    # (row|box) and (q|col) broadcasts are split into 3 slice-ops each.

    # q[f, r, bc] = row[f, r] | box[f, br(r), bc] — one op per ir
    q = wp.tile([128, F * 27], U16, tag="q")
    qv = q[:].rearrange("p (f b i k) -> p f b i k", f=F, b=3, i=3, k=3)
    rv3 = row[:].rearrange("p (f b i) -> p f b i", f=F, b=3, i=3)
    xv3 = box[:].rearrange("p (f b k) -> p f b k", f=F, b=3, k=3)
    for ir in range(3):
        nc.vector.tensor_tensor(
            qv[:, :, :, ir, :],
            rv3[:, :, :, ir].unsqueeze(3).broadcast_to((128, F, 3, 3)),
            xv3,
            op=OR,
        )

    # m[f, r, c] = q[f, r, bc(c)] | col[f, c] — one op per bc
    m = wp.tile([128, F * 81], U16, tag="m")
    mv = m[:].rearrange("p (f r b i) -> p f r b i", f=F, r=9, b=3, i=3)
    qv2 = q[:].rearrange("p (f r b) -> p f r b", f=F, r=9, b=3)
    cv2 = col[:].rearrange("p (f b i) -> p f b i", f=F, b=3, i=3)
    for bc in range(3):
        nc.vector.tensor_tensor(
            mv[:, :, :, bc, :],
            qv2[:, :, :, bc].unsqueeze(3).broadcast_to((128, F, 9, 3)),
            cv2[:, :, bc, :].unsqueeze(2).broadcast_to((128, F, 9, 3)),
            op=OR,
        )
    return m


def _entropy_from_m(nc, wp, m, e, F):
    n = F * 81
    h = wp.tile([128, n], U16, tag="h")
    nc.vector.tensor_scalar(h[:], m[:], 1, 0x155, op0=SHR, op1=AND)
    g = wp.tile([128, n], U16, tag="g")
    nc.vector.tensor_scalar(g[:], m[:], 2, 0x55, op0=SHR, op1=AND)
    s = wp.tile([128, n], U16, tag="s")
    (nc.gpsimd if GPSIMD_ADDS else nc.vector).tensor_tensor(s[:], h[:], g[:], op=ADD)

    a = wp.tile([128, n], U16, tag="a")
    nc.vector.tensor_scalar(a[:], s[:], 2, 0x33, op0=SHR, op1=AND)
    c0 = wp.tile([128, n], U16, tag="c0")
    nc.vector.tensor_scalar(c0[:], s[:], 0x333, None, op0=AND)
    c = wp.tile([128, n], U16, tag="c")
    (nc.gpsimd if GPSIMD_ADDS else nc.vector).tensor_tensor(c[:], c0[:], a[:], op=ADD)

    fw = wp.tile([128, n], U16, tag="fw")
    nc.vector.tensor_scalar(fw[:], e[:], 7, 8, op0=SHR, op1=AND)

    # pc = digit-sum of c's base-16 fields = c mod 15 (fields sum <= 9):
    #   q15 = floor(c/15)  (mult by 1/15, converted to int16)
    #   pc  = c - 15*q15
    # folded as t = (c - v) - fw with v = 15*q15.
    q15 = wp.tile([128, n], I16, tag="q15")
    nc.vector.tensor_scalar(q15[:], c[:], 1.0 / 15.0, MOD_BIAS, op0=MULT, op1=ADD)
    v15 = wp.tile([128, n], I16, tag="v15")
    nc.vector.tensor_scalar(v15[:], q15[:], 15, None, op0=MULT)
    t1 = wp.tile([128, n], I16, tag="t1")
    nc.vector.tensor_tensor(t1[:], c[:], v15[:], op=SUB)
    t = wp.tile([128, n], I16, tag="t")
    nc.vector.tensor_tensor(t[:], t1[:], fw[:], op=SUB)

    u = wp.tile([128, n], I16, tag="u")
    nc.vector.tensor_scalar(u[:], t[:], -1, 0, op0=MULT, op1=MAX)
    return u


def _emit(tc, out_ap, gb_ap, ga_ap, n_grids, F):
    nc = tc.nc
    per_tile = 128 * F
    n_tiles = n_grids // per_tile

    with ExitStack() as ctx:
        cp = ctx.enter_context(tc.tile_pool(name="const", bufs=1))
        iop = ctx.enter_context(tc.tile_pool(name="io", bufs=3))
        wp = ctx.enter_context(tc.tile_pool(name="work", bufs=4))
        accp = ctx.enter_context(tc.tile_pool(name="acc", bufs=3))

        enc_bias = cp.tile([128, 1], F32, tag="enc_bias")
        nc.vector.memset(enc_bias[:], LOG1024 + EPS)

        for i in range(n_tiles):
            tots = {}
            # Both Exp encodes first, then both Ln passes: halves the
            # ~1.3us InstLoadActFuncSet table reloads on ScalarE.
            encoded = {}
            for key, src in (("b", gb_ap), ("a", ga_ap)):
                x = iop.tile([128, F * 81], F32, tag="x")
                view = src[i * per_tile : (i + 1) * per_tile, :].rearrange(
                    "(p f) c -> p (f c)", p=128
                )
                nc.sync.dma_start(x[:], view)

                e = wp.tile([128, F * 81], U16, tag="e")
                nc.scalar.activation(
                    e[:],
                    x[:],
                    mybir.ActivationFunctionType.Exp,
                    bias=enc_bias[:],
                    scale=-LN2,
                )
                encoded[key] = e

            us = {}
            for key in ("b", "a"):
                e = encoded[key]
                m = _masks_and_m(nc, wp, e, F)
                us[key] = _entropy_from_m(nc, wp, m, e, F)

            for key in ("b", "a"):
                u = us[key]
                lnv = wp.tile([128, F * 81], F32, tag="lnv")
                nc.scalar.activation(
                    lnv[:], u[:], mybir.ActivationFunctionType.Ln, bias=1.0
                )

                tot = accp.tile([128, F], F32, tag="tot" + key)
                if GPSIMD_TREE:
                    lv = lnv[:].rearrange("p (f c) -> p f c", f=F, c=81)
                    a40 = wp.tile([128, F * 40], F32, tag="a40")
                    av = a40[:].rearrange("p (f c) -> p f c", f=F, c=40)
                    nc.gpsimd.tensor_tensor(av, lv[:, :, 0:40], lv[:, :, 40:80], op=ADD)
                    b20 = wp.tile([128, F * 20], F32, tag="b20")
                    bv = b20[:].rearrange("p (f c) -> p f c", f=F, c=20)
                    nc.gpsimd.tensor_tensor(bv, av[:, :, 0:20], av[:, :, 20:40], op=ADD)
                    c10 = wp.tile([128, F * 10], F32, tag="c10")
                    cv10 = c10[:].rearrange("p (f c) -> p f c", f=F, c=10)
                    nc.gpsimd.tensor_tensor(
                        cv10, bv[:, :, 0:10], bv[:, :, 10:20], op=ADD
                    )
                    d5 = wp.tile([128, F * 5], F32, tag="d5")
                    dv = d5[:].rearrange("p (f c) -> p f c", f=F, c=5)
                    nc.gpsimd.tensor_tensor(dv, cv10[:, :, 0:5], cv10[:, :, 5:10], op=ADD)
                    e2 = wp.tile([128, F * 2], F32, tag="e2")
                    ev = e2[:].rearrange("p (f c) -> p f c", f=F, c=2)
                    nc.gpsimd.tensor_tensor(ev, dv[:, :, 0:2], dv[:, :, 2:4], op=ADD)
                    f1 = wp.tile([128, F], F32, tag="f1t")
                    fv = f1[:].rearrange("p (f c) -> p f c", f=F, c=1)
                    nc.gpsimd.tensor_tensor(fv, ev[:, :, 0:1], ev[:, :, 1:2], op=ADD)
                    g1 = wp.tile([128, F], F32, tag="g1t")
                    gv1 = g1[:].rearrange("p (f c) -> p f c", f=F, c=1)
                    nc.gpsimd.tensor_tensor(gv1, fv, dv[:, :, 4:5], op=ADD)
                    tv = tot[:].rearrange("p (f c) -> p f c", f=F, c=1)
                    nc.gpsimd.tensor_tensor(tv, gv1, lv[:, :, 80:81], op=ADD)
                else:
                    nc.vector.tensor_reduce(
                        tot[:],
                        lnv[:].rearrange("p (f c) -> p f c", f=F, c=81),
                        axis=mybir.AxisListType.X,
                        op=ADD,
                    )
                tots[key] = tot

            diff = accp.tile([128, F], F32, tag="diff")
            nc.vector.tensor_tensor(diff[:], tots["b"][:], tots["a"][:], op=SUB)
            nc.vector.tensor_scalar(diff[:], diff[:], 1.0 / LN2, None, op0=MULT)
            out_view = out_ap[i * per_tile : (i + 1) * per_tile].rearrange(
                "(p f) -> p f", p=128
            )
            nc.sync.dma_start(out_view, diff[:])


_PROGRAM_CACHE = {}


def _build_program():
    key = (PER_CORE, F, USE_MOD, USE_GPSIMD, GPSIMD_TREE)
    if key in _PROGRAM_CACHE:
        return _PROGRAM_CACHE[key]
    nc = bacc.Bacc("TRN2", target_bir_lowering=False, debug=False)
    gb = nc.dram_tensor("grid_before", [PER_CORE, 81], F32, kind="ExternalInput")
    ga = nc.dram_tensor("grid_after", [PER_CORE, 81], F32, kind="ExternalInput")
    out = nc.dram_tensor("out", [PER_CORE], F32, kind="ExternalOutput")
    with tile.TileContext(nc) as tc:
        _emit(tc, out.ap(), gb.ap(), ga.ap(), PER_CORE, F)
    nc.finalize()
    _PROGRAM_CACHE[key] = nc
    return nc


def run(grid_before, grid_after, trace=False, **trace_kwargs):
    gb = np.ascontiguousarray(
        np.asarray(grid_before, dtype=np.float32).reshape(BATCH, 81)
    )
    ga = np.ascontiguousarray(
        np.asarray(grid_after, dtype=np.float32).reshape(BATCH, 81)
    )
    nc = _build_program()
    in_maps = [
        {
            "grid_before": gb[k * PER_CORE : (k + 1) * PER_CORE],
            "grid_after": ga[k * PER_CORE : (k + 1) * PER_CORE],
        }
        for k in range(N_CORES)
    ]
    res = run_bass_kernel_spmd(
        nc, in_maps, list(range(N_CORES)), trace=trace, **trace_kwargs
    )
    out = np.concatenate([res.results[k]["out"] for k in range(N_CORES)])
    return out, res


def kernel(grid_before, grid_after):
    out, _ = run(grid_before, grid_after)
    return out


def bench(grid_before, grid_after, iters=12, warmup=3):
    """Time repeated executions with device-resident inputs.

    Mirrors bass2jax.run_bass_via_pjrt's shard_map structure but keeps the
    170MB of inputs on the devices between iterations, so the measured
    per-iteration wall time approximates kernel execution + dispatch.
    """
    import time

    import jax
    import concourse.mybir as mybir_
    from jax.sharding import Mesh, NamedSharding, PartitionSpec
    from jax.experimental.shard_map import shard_map
    from concourse.bass2jax import (
        _bass_exec_p,
        install_neuronx_cc_hook,
        partition_id_tensor,
    )

    install_neuronx_cc_hook()
    gb = np.ascontiguousarray(
        np.asarray(grid_before, dtype=np.float32).reshape(BATCH, 81)
    )
    ga = np.ascontiguousarray(
        np.asarray(grid_after, dtype=np.float32).reshape(BATCH, 81)
    )
    nc = _build_program()

    part_name = nc.partition_id_tensor.name if nc.partition_id_tensor else None
    in_names, out_names, out_avals, zero_outs = [], [], [], []
    for alloc in nc.m.functions[0].allocations:
        if not isinstance(alloc, mybir.MemoryLocationSet):
            continue
        name = alloc.memorylocations[0].name
        if alloc.kind == "ExternalInput":
            if name != part_name:
                in_names.append(name)
        elif alloc.kind == "ExternalOutput":
            out_names.append(name)
            shape = tuple(alloc.tensor_shape)
            dtype = mybir_.dt.np(alloc.dtype)
            out_avals.append(jax.core.ShapedArray(shape, dtype))
            zero_outs.append(np.zeros((N_CORES * shape[0], *shape[1:]), dtype))
    n_params = len(in_names)
    all_names = in_names + out_names
    if part_name is not None:
        all_names = all_names + [part_name]

    def _body(*args):
        operands = list(args)
        if part_name is not None:
            operands.append(partition_id_tensor())
        outs = _bass_exec_p.bind(
            *operands,
            out_avals=tuple(out_avals),
            in_names=tuple(all_names),
            out_names=tuple(out_names),
            lowering_input_output_aliases=(),
            sim_require_finite=True,
            sim_require_nnan=True,
            nc=nc,
        )
        return tuple(outs)

    devices = jax.devices()[:N_CORES]
    mesh = Mesh(np.asarray(devices), ("core",))
    spec = NamedSharding(mesh, PartitionSpec("core"))
    sharded = jax.jit(
        shard_map(
            _body,
            mesh=mesh,
            in_specs=(PartitionSpec("core"),) * (n_params + len(out_names)),
            out_specs=(PartitionSpec("core"),) * len(out_names),
            check_rep=False,
        ),
        keep_unused=True,
    )
    host_in = {"grid_before": gb, "grid_after": ga}
    dev_in = [jax.device_put(host_in[nm], spec) for nm in in_names]
    dev_zero = [jax.device_put(z, spec) for z in zero_outs]

    for _ in range(warmup):
        outs = sharded(*dev_in, *dev_zero)
    jax.block_until_ready(outs)
    t0 = time.perf_counter()
    for _ in range(iters):
        outs = sharded(*dev_in, *dev_zero)
    jax.block_until_ready(outs)
    t1 = time.perf_counter()
    per_iter_ns = (t1 - t0) / iters * 1e9
    out = np.asarray(outs[0])
    return per_iter_ns, out



# revision 12
# speedup vs baseline: 1.0004x; 1.0004x over previous
"""Trainium2 Bass kernel: Sudoku information gain H(before) - H(after).

Self-contained: builds one SPMD Bass/Tile program, shards the batch
across 8 NeuronCores (pure data parallel), runs via
run_bass_kernel_spmd, and reassembles the full [B] output.

Algorithm per 9x9 grid (values 0..9, 0 = empty):
  encode each cell x as e = 1024 >> x on ScalarE (Exp activation):
    bit 10 <=> empty, bit (10-v) <=> value v.
  Bitwise-OR reductions of e give row/col/box presence masks (VectorE,
  uint16, 2x/4x-mode ops).  Per cell m = row|col|box.
  SWAR popcount: pair counts (h,g,s), base-16 fold (a,c0,c), then a
  single wrap-multiply digit-sum p = (c*0x1110)>>12 (top 4 bits hold
  the exact popcount; higher partial sums wrap off the u16).
  u = max(8*[empty] - p, 0), so Ln(u+1) = ln(max(9-forbidden,1)) for
  empty cells and 0 for filled ones (ScalarE Ln).  Per-grid add-reduce
  runs on the idle Pool engine; before/after are subtracted on-chip.

One explicit InstLoadActFuncSet pins the shared exp+ln activation
table, so the Exp/Ln mix causes no per-activation table reloads.
"""

import math
from contextlib import ExitStack

import numpy as np

import concourse.bass as bass
import concourse.bacc as bacc
import concourse.tile as tile
from concourse import mybir
from concourse.alu_op_type import AluOpType
from concourse.bass_utils import run_bass_kernel_spmd

F32 = mybir.dt.float32
U16 = mybir.dt.uint16
I16 = mybir.dt.int16

LN2 = math.log(2.0)
LOG1024 = math.log(1024.0)
EPS = 1e-5

OR = AluOpType.bitwise_or
AND = AluOpType.bitwise_and
ADD = AluOpType.add
SUB = AluOpType.subtract
MULT = AluOpType.mult
MAX = AluOpType.max
SHR = AluOpType.logical_shift_right

N_CORES = 8
BATCH = 262144
PER_CORE = BATCH // N_CORES  # 32768
F = 32  # grids per partition per tile
MOD_BIAS = -0.4999  # subtracted before the int16 convert in floor(c/15)
ACT_TABLE_BOTH = 6  # act_func_set_id of natural_log_exp_and_others


def _masks(nc, wp, e, F):
    """Per-cell forbidden mask m = row|col|box (u16, bits 1..9)."""
    ve = e[:]
    e4 = ve.rearrange("p (f r c) -> p f r c", f=F, r=9, c=9)
    e5 = ve.rearrange("p (f b i c) -> p f b i c", f=F, b=3, i=3, c=9)

    t3 = wp.tile([128, F * 27], U16, tag="t3")
    t3v = t3[:].rearrange("p (f r b) -> p f r b", f=F, r=9, b=3)
    nc.vector.tensor_tensor(t3v, e4[:, :, :, 0:3], e4[:, :, :, 3:6], op=OR)
    nc.vector.tensor_tensor(t3v, t3v, e4[:, :, :, 6:9], op=OR)

    row = wp.tile([128, F * 9], U16, tag="row")
    rv = row[:].rearrange("p (f r) -> p f r", f=F, r=9)
    t3b = t3[:].rearrange("p (f r b) -> p f r b", f=F, r=9, b=3)
    nc.vector.tensor_tensor(rv, t3b[:, :, :, 0], t3b[:, :, :, 1], op=OR)
    nc.vector.tensor_tensor(rv, rv, t3b[:, :, :, 2], op=OR)

    bcol = wp.tile([128, F * 27], U16, tag="bcol")
    bv = bcol[:].rearrange("p (f b c) -> p f b c", f=F, b=3, c=9)
    nc.vector.tensor_tensor(bv, e5[:, :, :, 0, :], e5[:, :, :, 1, :], op=OR)
    nc.vector.tensor_tensor(bv, bv, e5[:, :, :, 2, :], op=OR)

    col = wp.tile([128, F * 9], U16, tag="col")
    cv = col[:].rearrange("p (f c) -> p f c", f=F, c=9)
    bc3 = bcol[:].rearrange("p (f b c) -> p f b c", f=F, b=3, c=9)
    nc.vector.tensor_tensor(cv, bc3[:, :, 0, :], bc3[:, :, 1, :], op=OR)
    nc.vector.tensor_tensor(cv, cv, bc3[:, :, 2, :], op=OR)

    box = wp.tile([128, F * 9], U16, tag="box")
    xv = box[:].rearrange("p (f b k) -> p f b k", f=F, b=3, k=3)
    bc4 = bcol[:].rearrange("p (f b k i) -> p f b k i", f=F, b=3, k=3, i=3)
    nc.vector.tensor_tensor(xv, bc4[:, :, :, :, 0], bc4[:, :, :, :, 1], op=OR)
    nc.vector.tensor_tensor(xv, xv, bc4[:, :, :, :, 2], op=OR)

    # q[f, r, bc] = row[f, r] | box[f, br(r), bc] — one op per ir
    # (tensor ops allow at most 3 free dims per AP, so the broadcasts
    # are split into 3 slice-ops each)
    q = wp.tile([128, F * 27], U16, tag="q")
    qv = q[:].rearrange("p (f b i k) -> p f b i k", f=F, b=3, i=3, k=3)
    rv3 = row[:].rearrange("p (f b i) -> p f b i", f=F, b=3, i=3)
    xv3 = box[:].rearrange("p (f b k) -> p f b k", f=F, b=3, k=3)
    for ir in range(3):
        nc.vector.tensor_tensor(
            qv[:, :, :, ir, :],
            rv3[:, :, :, ir].unsqueeze(3).broadcast_to((128, F, 3, 3)),
            xv3,
            op=OR,
        )

    # qx[f, r, c] = q[f, r, bc(c)]: replicate q 3x along the innermost
    # cell axis on the idle Activation engine (stride-0 reads are legal
    # there, unlike DVE 2x mode or DMA).  Then m = qx | col is a single
    # full-width DVE op with every operand packed (2x mode) instead of
    # three 1x broadcast ops.
    qx = wp.tile([128, F * 81], U16, tag="qx")
    qxv = qx[:].rearrange("p (u i) -> p u i", u=F * 27, i=3)
    qu = q[:].rearrange("p (u) -> p u", u=F * 27)
    nc.scalar.copy(qxv, qu.unsqueeze(2).broadcast_to((128, F * 27, 3)))

    m = wp.tile([128, F * 81], U16, tag="m")
    mv3 = m[:].rearrange("p (f r c) -> p f r c", f=F, r=9, c=9)
    colb = col[:].rearrange("p (f c) -> p f c", f=F, c=9)
    nc.vector.tensor_tensor(
        mv3,
        qx[:].rearrange("p (f r c) -> p f r c", f=F, r=9, c=9),
        colb.unsqueeze(2).broadcast_to((128, F, 9, 9)),
        op=OR,
    )
    return m


def _entropy_u(nc, wp, m, e, F):
    """u(i16) with Ln(u+1) = per-cell entropy contribution.

    SWAR popcount of m's bits 1..9 (bit 10 never enters: the 0x155/0x55
    masks skip it), mod-15 digit sum (u16 multiply saturates on TRN2 so
    the wrap-multiply shortcut is unusable), fused with the own-cell
    empty gate from e's bit 10.  Buffers A/B/C and m are reused in
    place across chain stages.
    """
    n = F * 81
    A = wp.tile([128, n], U16, tag="A")
    B = wp.tile([128, n], U16, tag="B")
    C = wp.tile([128, n], U16, tag="C")

    h, g = A, B
    nc.vector.tensor_scalar(h[:], m[:], 1, 0x155, op0=SHR, op1=AND)
    nc.vector.tensor_scalar(g[:], m[:], 2, 0x55, op0=SHR, op1=AND)
    s = m  # m dead after h,g
    nc.vector.tensor_tensor(s[:], h[:], g[:], op=ADD)
    a2, c0 = A, B  # h,g consumed
    nc.vector.tensor_scalar(a2[:], s[:], 2, 0x33, op0=SHR, op1=AND)
    nc.vector.tensor_scalar(c0[:], s[:], 0x333, None, op0=AND)
    c = C
    nc.vector.tensor_tensor(c[:], c0[:], a2[:], op=ADD)
    # c = f0 + 16*f1 + 256*f2 with f0,f1<=4, f2<=1; popcount = c mod 15
    q15 = A[:].bitcast(I16)  # a2 consumed
    nc.vector.tensor_scalar(q15, c[:], 1.0 / 15.0, MOD_BIAS, op0=MULT, op1=ADD)
    v15 = B[:].bitcast(I16)  # c0 consumed
    nc.vector.tensor_scalar(v15, q15, 15, None, op0=MULT)
    t1 = m[:].bitcast(I16)  # s dead after a2,c0
    nc.vector.tensor_tensor(t1, c[:], v15, op=SUB)
    fw = C  # c consumed by t1; 8*[cell empty]
    nc.vector.tensor_scalar(fw[:], e[:], 7, 8, op0=SHR, op1=AND)
    t = A[:].bitcast(I16)  # q15 dead
    nc.vector.tensor_tensor(t, t1, fw[:], op=SUB)
    u = B[:].bitcast(I16)  # v15 dead
    nc.vector.tensor_scalar(u, t, -1, 0, op0=MULT, op1=MAX)
    return u


def _emit(tc, out_ap, gb_ap, ga_ap, n_grids, F):
    nc = tc.nc
    per_tile = 128 * F
    n_tiles = n_grids // per_tile

    # Pin the activation table that contains BOTH exp and ln: without
    # this the table-load pass alternates exp/ln tables (1.3us each).
    ld = mybir.InstLoadActFuncSet(
        name=nc.get_next_instruction_name(),
        act_func_set_id=ACT_TABLE_BOTH,
        ins=[],
        outs=[],
    )
    nc.scalar.add_instruction(ld)

    with ExitStack() as ctx:
        cp = ctx.enter_context(tc.tile_pool(name="const", bufs=1))
        iop = ctx.enter_context(tc.tile_pool(name="io", bufs=4))
        wp = ctx.enter_context(tc.tile_pool(name="work", bufs=3))
        accp = ctx.enter_context(tc.tile_pool(name="acc", bufs=3))

        enc_bias = cp.tile([128, 1], F32, tag="enc_bias")
        nc.vector.memset(enc_bias[:], LOG1024 + EPS)

        for i in range(n_tiles):
            xs = {}
            encoded = {}
            for key, src in (("b", gb_ap), ("a", ga_ap)):
                x = iop.tile([128, F * 81], F32, tag="x")
                view = src[i * per_tile : (i + 1) * per_tile, :].rearrange(
                    "(p f) c -> p (f c)", p=128
                )
                nc.sync.dma_start(x[:], view)

                e = wp.tile([128, F * 81], U16, tag="e")
                nc.scalar.activation(
                    e[:],
                    x[:],
                    mybir.ActivationFunctionType.Exp,
                    bias=enc_bias[:],
                    scale=-LN2,
                )
                xs[key] = x
                encoded[key] = e

            tots = {}
            for key in ("b", "a"):
                e = encoded[key]
                m = _masks(nc, wp, e, F)
                u = _entropy_u(nc, wp, m, e, F)

                lnv = xs[key]  # reuse the f32 input buffer for Ln output
                nc.scalar.activation(
                    lnv[:], u, mybir.ActivationFunctionType.Ln, bias=1.0
                )

                # Per-grid sum of the 81 ln values entirely on the idle
                # Pool engine: in-place 81->27->9->3->1 fold tree inside
                # the lnv buffer (Pool only supports f32 arithmetic).
                lv = lnv[:].rearrange("p (f c) -> p f c", f=F, c=81)
                for width in (27, 9, 3, 1):
                    nc.gpsimd.tensor_tensor(
                        lv[:, :, 0:width],
                        lv[:, :, 0:width],
                        lv[:, :, width : 2 * width],
                        op=ADD,
                    )
                    nc.gpsimd.tensor_tensor(
                        lv[:, :, 0:width],
                        lv[:, :, 0:width],
                        lv[:, :, 2 * width : 3 * width],
                        op=ADD,
                    )
                tots[key] = lv[:, :, 0]

            diff = accp.tile([128, F], F32, tag="diff")
            nc.gpsimd.tensor_tensor(diff[:], tots["b"], tots["a"], op=SUB)
            nc.gpsimd.tensor_scalar(diff[:], diff[:], 1.0 / LN2, None, op0=MULT)
            out_view = out_ap[i * per_tile : (i + 1) * per_tile].rearrange(
                "(p f) -> p f", p=128
            )
            nc.sync.dma_start(out_view, diff[:])


_PROGRAM_CACHE = {}


def _build_program():
    key = (PER_CORE, F)
    if key in _PROGRAM_CACHE:
        return _PROGRAM_CACHE[key]
    nc = bacc.Bacc("TRN2", target_bir_lowering=False, debug=False)
    gb = nc.dram_tensor("grid_before", [PER_CORE, 81], F32, kind="ExternalInput")
    ga = nc.dram_tensor("grid_after", [PER_CORE, 81], F32, kind="ExternalInput")
    out = nc.dram_tensor("out", [PER_CORE], F32, kind="ExternalOutput")
    with tile.TileContext(nc) as tc:
        _emit(tc, out.ap(), gb.ap(), ga.ap(), PER_CORE, F)
    nc.finalize()
    _PROGRAM_CACHE[key] = nc
    return nc


def run(grid_before, grid_after, trace=False, **trace_kwargs):
    gb = np.ascontiguousarray(
        np.asarray(grid_before, dtype=np.float32).reshape(BATCH, 81)
    )
    ga = np.ascontiguousarray(
        np.asarray(grid_after, dtype=np.float32).reshape(BATCH, 81)
    )
    nc = _build_program()
    in_maps = [
        {
            "grid_before": gb[k * PER_CORE : (k + 1) * PER_CORE],
            "grid_after": ga[k * PER_CORE : (k + 1) * PER_CORE],
        }
        for k in range(N_CORES)
    ]
    res = run_bass_kernel_spmd(
        nc, in_maps, list(range(N_CORES)), trace=trace, **trace_kwargs
    )
    out = np.concatenate([res.results[k]["out"] for k in range(N_CORES)])
    return out, res


def kernel(grid_before, grid_after):
    out, _ = run(grid_before, grid_after)
    return out


def bench(grid_before, grid_after, iters=12, warmup=3):
    """Time repeated executions with device-resident inputs.

    Mirrors bass2jax.run_bass_via_pjrt's shard_map structure but keeps the
    170MB of inputs on the devices between iterations, so the measured
    per-iteration wall time approximates kernel execution + dispatch.
    """
    import time

    import jax
    import concourse.mybir as mybir_
    from jax.sharding import Mesh, NamedSharding, PartitionSpec
    from jax.experimental.shard_map import shard_map
    from concourse.bass2jax import (
        _bass_exec_p,
        install_neuronx_cc_hook,
        partition_id_tensor,
    )

    install_neuronx_cc_hook()
    gb = np.ascontiguousarray(
        np.asarray(grid_before, dtype=np.float32).reshape(BATCH, 81)
    )
    ga = np.ascontiguousarray(
        np.asarray(grid_after, dtype=np.float32).reshape(BATCH, 81)
    )
    nc = _build_program()

    part_name = nc.partition_id_tensor.name if nc.partition_id_tensor else None
    in_names, out_names, out_avals, zero_outs = [], [], [], []
    for alloc in nc.m.functions[0].allocations:
        if not isinstance(alloc, mybir.MemoryLocationSet):
            continue
        name = alloc.memorylocations[0].name
        if alloc.kind == "ExternalInput":
            if name != part_name:
                in_names.append(name)
        elif alloc.kind == "ExternalOutput":
            out_names.append(name)
            shape = tuple(alloc.tensor_shape)
            dtype = mybir_.dt.np(alloc.dtype)
            out_avals.append(jax.core.ShapedArray(shape, dtype))
            zero_outs.append(np.zeros((N_CORES * shape[0], *shape[1:]), dtype))
    n_params = len(in_names)
    all_names = in_names + out_names
    if part_name is not None:
        all_names = all_names + [part_name]

    def _body(*args):
        operands = list(args)
        if part_name is not None:
            operands.append(partition_id_tensor())
        outs = _bass_exec_p.bind(
            *operands,
            out_avals=tuple(out_avals),
            in_names=tuple(all_names),
            out_names=tuple(out_names),
            lowering_input_output_aliases=(),
            sim_require_finite=True,
            sim_require_nnan=True,
            nc=nc,
        )
        return tuple(outs)

    devices = jax.devices()[:N_CORES]
    mesh = Mesh(np.asarray(devices), ("core",))
    spec = NamedSharding(mesh, PartitionSpec("core"))
    sharded = jax.jit(
        shard_map(
            _body,
            mesh=mesh,
            in_specs=(PartitionSpec("core"),) * (n_params + len(out_names)),
            out_specs=(PartitionSpec("core"),) * len(out_names),
            check_rep=False,
        ),
        keep_unused=True,
    )
    host_in = {"grid_before": gb, "grid_after": ga}
    dev_in = [jax.device_put(host_in[nm], spec) for nm in in_names]
    dev_zero = [jax.device_put(z, spec) for z in zero_outs]

    for _ in range(warmup):
        outs = sharded(*dev_in, *dev_zero)
    jax.block_until_ready(outs)
    t0 = time.perf_counter()
    for _ in range(iters):
        outs = sharded(*dev_in, *dev_zero)
    jax.block_until_ready(outs)
    t1 = time.perf_counter()
    per_iter_ns = (t1 - t0) / iters * 1e9
    out = np.asarray(outs[0])
    return per_iter_ns, out


# revision 18
# speedup vs baseline: 1.0265x; 1.0261x over previous
"""Trainium2 Bass kernel: Sudoku information gain H(before) - H(after).

Self-contained: builds one SPMD Bass/Tile program, shards the batch
across 8 NeuronCores (pure data parallel), runs via
run_bass_kernel_spmd, and reassembles the full [B] output.

Algorithm per 9x9 grid (values 0..9, 0 = empty):
  encode each cell x as e = 1024 >> x on ScalarE (Exp activation):
    bit 10 <=> empty, bit (10-v) <=> value v.
  Bitwise-OR reductions of e give row/col/box presence masks (VectorE,
  uint16, 2x/4x-mode ops).  Per cell m = row|col|box.
  SWAR popcount: pair counts (h,g,s), base-16 fold (a,c0,c), then a
  single wrap-multiply digit-sum p = (c*0x1110)>>12 (top 4 bits hold
  the exact popcount; higher partial sums wrap off the u16).
  u = max(8*[empty] - p, 0), so Ln(u+1) = ln(max(9-forbidden,1)) for
  empty cells and 0 for filled ones (ScalarE Ln).  Per-grid add-reduce
  runs on the idle Pool engine; before/after are subtracted on-chip.

One explicit InstLoadActFuncSet pins the shared exp+ln activation
table, so the Exp/Ln mix causes no per-activation table reloads.
"""

import math
from contextlib import ExitStack

import numpy as np

import concourse.bass as bass
import concourse.bacc as bacc
import concourse.tile as tile
from concourse import mybir
from concourse.alu_op_type import AluOpType
from concourse.bass_utils import run_bass_kernel_spmd

F32 = mybir.dt.float32
U16 = mybir.dt.uint16
I16 = mybir.dt.int16

LN2 = math.log(2.0)
LOG1024 = math.log(1024.0)
EPS = 1e-5

OR = AluOpType.bitwise_or
AND = AluOpType.bitwise_and
ADD = AluOpType.add
SUB = AluOpType.subtract
MULT = AluOpType.mult
MAX = AluOpType.max
SHR = AluOpType.logical_shift_right

N_CORES = 8
BATCH = 262144
PER_CORE = BATCH // N_CORES  # 32768
F = 32  # grids per partition per tile
MOD_BIAS = -0.4999  # subtracted before the int16 convert in floor(c/15)
ACT_TABLE_BOTH = 6  # act_func_set_id of natural_log_exp_and_others


def _masks(nc, wp, e, F):
    """Per-cell forbidden mask m = row|col|box (u16, bits 1..9)."""
    ve = e[:]
    e4 = ve.rearrange("p (f r c) -> p f r c", f=F, r=9, c=9)
    e5 = ve.rearrange("p (f b i c) -> p f b i c", f=F, b=3, i=3, c=9)

    t3 = wp.tile([128, F * 27], U16, tag="t3")
    t3v = t3[:].rearrange("p (f r b) -> p f r b", f=F, r=9, b=3)
    nc.vector.tensor_tensor(t3v, e4[:, :, :, 0:3], e4[:, :, :, 3:6], op=OR)
    nc.vector.tensor_tensor(t3v, t3v, e4[:, :, :, 6:9], op=OR)

    # transpose t3 (f,r,b)->(f,b,r) on the Activation engine so the
    # row-OR reads packed slices (DVE 2x mode) instead of stride-3 ones
    t3t = wp.tile([128, F * 27], U16, tag="t3t")
    t3tv = t3t[:].rearrange("p (f b r) -> p f b r", f=F, b=3, r=9)
    nc.scalar.copy(t3tv, t3[:].rearrange("p (f r b) -> p f b r", f=F, r=9, b=3))

    row = wp.tile([128, F * 9], U16, tag="row")
    rv = row[:].rearrange("p (f r) -> p f r", f=F, r=9)
    nc.vector.tensor_tensor(rv, t3tv[:, :, 0, :], t3tv[:, :, 1, :], op=OR)
    nc.vector.tensor_tensor(rv, rv, t3tv[:, :, 2, :], op=OR)

    bcol = wp.tile([128, F * 27], U16, tag="bcol")
    bv = bcol[:].rearrange("p (f b c) -> p f b c", f=F, b=3, c=9)
    nc.vector.tensor_tensor(bv, e5[:, :, :, 0, :], e5[:, :, :, 1, :], op=OR)
    nc.vector.tensor_tensor(bv, bv, e5[:, :, :, 2, :], op=OR)

    col = wp.tile([128, F * 9], U16, tag="col")
    cv = col[:].rearrange("p (f c) -> p f c", f=F, c=9)
    bc3 = bcol[:].rearrange("p (f b c) -> p f b c", f=F, b=3, c=9)
    nc.vector.tensor_tensor(cv, bc3[:, :, 0, :], bc3[:, :, 1, :], op=OR)
    nc.vector.tensor_tensor(cv, cv, bc3[:, :, 2, :], op=OR)

    # transpose bcol's (k,i) cell split on Act so the box-OR is packed
    bct = wp.tile([128, F * 27], U16, tag="bct")
    bctv = bct[:].rearrange("p (g i k) -> p g i k", g=F * 3, i=3, k=3)
    nc.scalar.copy(bctv, bcol[:].rearrange("p (g k i) -> p g i k", g=F * 3, k=3, i=3))

    box = wp.tile([128, F * 9], U16, tag="box")
    xv = box[:].rearrange("p (g k) -> p g k", g=F * 3, k=3)
    nc.vector.tensor_tensor(xv, bctv[:, :, 0, :], bctv[:, :, 1, :], op=OR)
    nc.vector.tensor_tensor(xv, xv, bctv[:, :, 2, :], op=OR)

    # rowx[f, (b,i), k] = row[f, (b,i)]: replicate on Act (stride-0)
    rowx = wp.tile([128, F * 27], U16, tag="rowx")
    rowxv = rowx[:].rearrange("p (u k) -> p u k", u=F * 9, k=3)
    nc.scalar.copy(rowxv, row[:].unsqueeze(2).broadcast_to((128, F * 9, 3)))

    # q[f, (b,i), bc] = rowx | box (box bcast over i sits on a middle
    # dim, so every AP keeps a packed innermost -> 2x mode)
    q = wp.tile([128, F * 27], U16, tag="q")
    qv = q[:].rearrange("p (g i k) -> p g i k", g=F * 3, i=3, k=3)
    nc.vector.tensor_tensor(
        qv,
        rowx[:].rearrange("p (g i k) -> p g i k", g=F * 3, i=3, k=3),
        xv.unsqueeze(2).broadcast_to((128, F * 3, 3, 3)),
        op=OR,
    )

    # qx[f, r, c] = q[f, r, bc(c)]: replicate q 3x along the innermost
    # cell axis on the idle Activation engine (stride-0 reads are legal
    # there, unlike DVE 2x mode or DMA).  Then m = qx | col is a single
    # full-width DVE op with every operand packed (2x mode) instead of
    # three 1x broadcast ops.
    qx = wp.tile([128, F * 81], U16, tag="qx")
    qxv = qx[:].rearrange("p (u i) -> p u i", u=F * 27, i=3)
    qu = q[:].rearrange("p (u) -> p u", u=F * 27)
    nc.scalar.copy(qxv, qu.unsqueeze(2).broadcast_to((128, F * 27, 3)))
    return qx, col


def _combine(nc, wp, qx, col, F):
    m = wp.tile([128, F * 81], U16, tag="m", bufs=2)
    mv3 = m[:].rearrange("p (f r c) -> p f r c", f=F, r=9, c=9)
    colb = col[:].rearrange("p (f c) -> p f c", f=F, c=9)
    nc.vector.tensor_tensor(
        mv3,
        qx[:].rearrange("p (f r c) -> p f r c", f=F, r=9, c=9),
        colb.unsqueeze(2).broadcast_to((128, F, 9, 9)),
        op=OR,
    )
    return m


def _entropy_u(nc, wp, m, e, F):
    """u(i16) with Ln(u+1) = per-cell entropy contribution.

    SWAR popcount of m's bits 1..9 (bit 10 never enters: the 0x155/0x55
    masks skip it), mod-15 digit sum (u16 multiply saturates on TRN2 so
    the wrap-multiply shortcut is unusable), fused with the own-cell
    empty gate from e's bit 10.  Buffers A/B/C and m are reused in
    place across chain stages.
    """
    n = F * 81
    A = wp.tile([128, n], U16, tag="A", bufs=2)
    B = wp.tile([128, n], U16, tag="B", bufs=2)
    C = wp.tile([128, n], U16, tag="C", bufs=2)

    h, g = A, B
    nc.vector.tensor_scalar(h[:], m[:], 1, 0x155, op0=SHR, op1=AND)
    nc.vector.tensor_scalar(g[:], m[:], 2, 0x55, op0=SHR, op1=AND)
    s = m  # m dead after h,g
    nc.vector.tensor_tensor(s[:], h[:], g[:], op=ADD)
    a2, c0 = A, B  # h,g consumed
    nc.vector.tensor_scalar(a2[:], s[:], 2, 0x33, op0=SHR, op1=AND)
    nc.vector.tensor_scalar(c0[:], s[:], 0x333, None, op0=AND)
    c = C
    nc.vector.tensor_tensor(c[:], c0[:], a2[:], op=ADD)
    # c = f0 + 16*f1 + 256*f2 with f0,f1<=4, f2<=1; popcount = c mod 15
    q15 = A[:].bitcast(I16)  # a2 consumed
    nc.vector.tensor_scalar(q15, c[:], 1.0 / 15.0, MOD_BIAS, op0=MULT, op1=ADD)
    v15 = B[:].bitcast(I16)  # c0 consumed
    nc.vector.tensor_scalar(v15, q15, 15, None, op0=MULT)
    t1 = m[:].bitcast(I16)  # s dead after a2,c0
    nc.vector.tensor_tensor(t1, c[:], v15, op=SUB)
    fw = C  # c consumed by t1; 8*[cell empty]
    nc.vector.tensor_scalar(fw[:], e[:], 7, 8, op0=SHR, op1=AND)
    t = A[:].bitcast(I16)  # q15 dead
    nc.vector.tensor_tensor(t, t1, fw[:], op=SUB)
    u = B[:].bitcast(I16)  # v15 dead
    nc.vector.tensor_scalar(u, t, -1, 0, op0=MULT, op1=MAX)
    return u


def _emit(tc, out_ap, gb_ap, ga_ap, n_grids, F):
    nc = tc.nc
    per_tile = 128 * F
    n_tiles = n_grids // per_tile

    # Pin the activation table that contains BOTH exp and ln: without
    # this the table-load pass alternates exp/ln tables (1.3us each).
    ld = mybir.InstLoadActFuncSet(
        name=nc.get_next_instruction_name(),
        act_func_set_id=ACT_TABLE_BOTH,
        ins=[],
        outs=[],
    )
    nc.scalar.add_instruction(ld)

    with ExitStack() as ctx:
        cp = ctx.enter_context(tc.tile_pool(name="const", bufs=1))
        iop = ctx.enter_context(tc.tile_pool(name="io", bufs=3))
        wp = ctx.enter_context(tc.tile_pool(name="work", bufs=4))
        accp = ctx.enter_context(tc.tile_pool(name="acc", bufs=3))

        enc_bias = cp.tile([128, 1], F32, tag="enc_bias")
        nc.vector.memset(enc_bias[:], LOG1024 + EPS)

        state = {}

        def pre(i):
            """DMA + encode + mask build through qx for tile i."""
            st = {}
            for key, src in (("b", gb_ap), ("a", ga_ap)):
                x = iop.tile([128, F * 81], F32, tag="x")
                view = src[i * per_tile : (i + 1) * per_tile, :].rearrange(
                    "(p f) c -> p (f c)", p=128
                )
                nc.sync.dma_start(x[:], view)
                e = wp.tile([128, F * 81], U16, tag="e")
                nc.scalar.activation(
                    e[:],
                    x[:],
                    mybir.ActivationFunctionType.Exp,
                    bias=enc_bias[:],
                    scale=-LN2,
                )
                st[key] = (e, _masks(nc, wp, e, F))
            state[i] = st

        def main(i):
            """Per-cell mask | col, SWAR chain, Ln, Pool fold for tile i."""
            st = state.pop(i)
            tots = {}
            for key in ("b", "a"):
                e, (qx, col) = st[key]
                m = _combine(nc, wp, qx, col, F)
                u = _entropy_u(nc, wp, m, e, F)
                lnv = wp.tile([128, F * 81], F32, tag="lnv", bufs=2)
                nc.scalar.activation(
                    lnv[:], u, mybir.ActivationFunctionType.Ln, bias=1.0
                )
                # Per-grid sum of the 81 ln values entirely on the idle
                # Pool engine: in-place 81->27->9->3->1 fold tree (Pool
                # only supports f32 arithmetic).
                lv = lnv[:].rearrange("p (f c) -> p f c", f=F, c=81)
                for width in (27, 9, 3, 1):
                    nc.gpsimd.tensor_tensor(
                        lv[:, :, 0:width],
                        lv[:, :, 0:width],
                        lv[:, :, width : 2 * width],
                        op=ADD,
                    )
                    nc.gpsimd.tensor_tensor(
                        lv[:, :, 0:width],
                        lv[:, :, 0:width],
                        lv[:, :, 2 * width : 3 * width],
                        op=ADD,
                    )
                tots[key] = lv[:, :, 0]

            diff = accp.tile([128, F], F32, tag="diff")
            nc.gpsimd.tensor_tensor(diff[:], tots["b"], tots["a"], op=SUB)
            nc.gpsimd.tensor_scalar(diff[:], diff[:], 1.0 / LN2, None, op0=MULT)
            out_view = out_ap[i * per_tile : (i + 1) * per_tile].rearrange(
                "(p f) -> p f", p=128
            )
            nc.sync.dma_start(out_view, diff[:])

        # one-tile software pipeline skew: tile i's cross-engine mask
        # staging (Act transposes/replications) completes while the DVE
        # drains tile i-1's long chain, so the in-order DVE queue never
        # stalls on the Activation engine.
        for i in range(n_tiles + 1):
            if i < n_tiles:
                pre(i)
            if i >= 1:
                main(i - 1)


_PROGRAM_CACHE = {}


def _build_program():
    key = (PER_CORE, F)
    if key in _PROGRAM_CACHE:
        return _PROGRAM_CACHE[key]
    nc = bacc.Bacc("TRN2", target_bir_lowering=False, debug=False)
    gb = nc.dram_tensor("grid_before", [PER_CORE, 81], F32, kind="ExternalInput")
    ga = nc.dram_tensor("grid_after", [PER_CORE, 81], F32, kind="ExternalInput")
    out = nc.dram_tensor("out", [PER_CORE], F32, kind="ExternalOutput")
    with tile.TileContext(nc) as tc:
        _emit(tc, out.ap(), gb.ap(), ga.ap(), PER_CORE, F)
    nc.finalize()
    _PROGRAM_CACHE[key] = nc
    return nc


def run(grid_before, grid_after, trace=False, **trace_kwargs):
    gb = np.ascontiguousarray(
        np.asarray(grid_before, dtype=np.float32).reshape(BATCH, 81)
    )
    ga = np.ascontiguousarray(
        np.asarray(grid_after, dtype=np.float32).reshape(BATCH, 81)
    )
    nc = _build_program()
    in_maps = [
        {
            "grid_before": gb[k * PER_CORE : (k + 1) * PER_CORE],
            "grid_after": ga[k * PER_CORE : (k + 1) * PER_CORE],
        }
        for k in range(N_CORES)
    ]
    res = run_bass_kernel_spmd(
        nc, in_maps, list(range(N_CORES)), trace=trace, **trace_kwargs
    )
    out = np.concatenate([res.results[k]["out"] for k in range(N_CORES)])
    return out, res


def kernel(grid_before, grid_after):
    out, _ = run(grid_before, grid_after)
    return out


def bench(grid_before, grid_after, iters=12, warmup=3):
    """Time repeated executions with device-resident inputs.

    Mirrors bass2jax.run_bass_via_pjrt's shard_map structure but keeps the
    170MB of inputs on the devices between iterations, so the measured
    per-iteration wall time approximates kernel execution + dispatch.
    """
    import time

    import jax
    import concourse.mybir as mybir_
    from jax.sharding import Mesh, NamedSharding, PartitionSpec
    from jax.experimental.shard_map import shard_map
    from concourse.bass2jax import (
        _bass_exec_p,
        install_neuronx_cc_hook,
        partition_id_tensor,
    )

    install_neuronx_cc_hook()
    gb = np.ascontiguousarray(
        np.asarray(grid_before, dtype=np.float32).reshape(BATCH, 81)
    )
    ga = np.ascontiguousarray(
        np.asarray(grid_after, dtype=np.float32).reshape(BATCH, 81)
    )
    nc = _build_program()

    part_name = nc.partition_id_tensor.name if nc.partition_id_tensor else None
    in_names, out_names, out_avals, zero_outs = [], [], [], []
    for alloc in nc.m.functions[0].allocations:
        if not isinstance(alloc, mybir.MemoryLocationSet):
            continue
        name = alloc.memorylocations[0].name
        if alloc.kind == "ExternalInput":
            if name != part_name:
                in_names.append(name)
        elif alloc.kind == "ExternalOutput":
            out_names.append(name)
            shape = tuple(alloc.tensor_shape)
            dtype = mybir_.dt.np(alloc.dtype)
            out_avals.append(jax.core.ShapedArray(shape, dtype))
            zero_outs.append(np.zeros((N_CORES * shape[0], *shape[1:]), dtype))
    n_params = len(in_names)
    all_names = in_names + out_names
    if part_name is not None:
        all_names = all_names + [part_name]

    def _body(*args):
        operands = list(args)
        if part_name is not None:
            operands.append(partition_id_tensor())
        outs = _bass_exec_p.bind(
            *operands,
            out_avals=tuple(out_avals),
            in_names=tuple(all_names),
            out_names=tuple(out_names),
            lowering_input_output_aliases=(),
            sim_require_finite=True,
            sim_require_nnan=True,
            nc=nc,
        )
        return tuple(outs)

    devices = jax.devices()[:N_CORES]
    mesh = Mesh(np.asarray(devices), ("core",))
    spec = NamedSharding(mesh, PartitionSpec("core"))
    sharded = jax.jit(
        shard_map(
            _body,
            mesh=mesh,
            in_specs=(PartitionSpec("core"),) * (n_params + len(out_names)),
            out_specs=(PartitionSpec("core"),) * len(out_names),
            check_rep=False,
        ),
        keep_unused=True,
    )
    host_in = {"grid_before": gb, "grid_after": ga}
    dev_in = [jax.device_put(host_in[nm], spec) for nm in in_names]
    dev_zero = [jax.device_put(z, spec) for z in zero_outs]

    for _ in range(warmup):
        outs = sharded(*dev_in, *dev_zero)
    jax.block_until_ready(outs)
    t0 = time.perf_counter()
    for _ in range(iters):
        outs = sharded(*dev_in, *dev_zero)
    jax.block_until_ready(outs)
    t1 = time.perf_counter()
    per_iter_ns = (t1 - t0) / iters * 1e9
    out = np.asarray(outs[0])
    return per_iter_ns, out
